# revision 1
# baseline (speedup 1.0000x reference)
"""DAGNN propagation kernel for 8 Trainium2 NeuronCores.

Strategy: partition nodes across the 8 cores (12,500 nodes each). Each hop is
an SpMM h_next = A_norm @ h computed per core for its own nodes:
  - the full scaled feature table (h * src_norm) is replicated to every core
    by an AllGather at the end of the previous hop
  - per edge, the source row is fetched with dma_gather (512B rows)
  - the scatter-add over destinations is a sequence of one-hot matmuls
    accumulating into a PSUM tile per 128-node destination bin; one-hot
    matrices are built on-chip by comparing an iota row against the edge's
    destination offset (broadcast access patterns, no materialized data)
  - the DAGNN gate (sigmoid(h_k . s) * h_k) is accumulated per bin right
    after each hop so hop results never round-trip through HBM
Host-side numpy does the graph preprocessing once: degree norms, edge
partitioning by (destination bin, source table chunk), padding to 128-edge
chunklets uniform across cores (SPMD requires one program for all cores).
"""

import sys
import numpy as np

for _p in ("/root/.axon_site", "/root/.axon_site/_ro/trn_rl_repo", "/opt/trn_rl_repo"):
    if _p not in sys.path:
        sys.path.append(_p)

N = 100000
D = 128
K = 10
C = 8
SH = 12500          # nodes per core
NB = 98             # destination bins per core (ceil(12500/128))
SHP = NB * 128      # padded shard rows = 12544
TBL = C * SHP       # gathered table rows = 100352
NQ = 4
CH = TBL // NQ      # 25088 rows per gather chunk (int16-addressable)
SG = 4              # bins per supergroup
FILL_OFF = 200.0    # dstoff sentinel for padding edges -> all-zero one-hot row


def _wrap_idx(arr):
    """int16 index array [n] (n % 128 == 0) -> dma_gather wrapped layout
    [128, n//16]: logical i at [i % 16, i // 16], replicated to all 8 Q7
    cores (partitions 16r + i%16)."""
    n = arr.shape[0]
    w = arr.reshape(n // 16, 16).T  # [16, n//16]
    return np.tile(w, (8, 1)).astype(np.int16)


def preprocess(feats, s, src, dst):
    feats = np.asarray(feats, dtype=np.float32)
    s = np.asarray(s, dtype=np.float32).reshape(D)
    src = np.asarray(src, dtype=np.int64)
    dst = np.asarray(dst, dtype=np.int64)

    in_deg = np.bincount(dst, minlength=N).astype(np.float32)
    out_deg = np.bincount(src, minlength=N).astype(np.float32)
    dst_norm = np.clip(in_deg, 1.0, None) ** -0.5
    src_norm = np.clip(out_deg, 1.0, None) ** -0.5

    owner = src // SH
    table_row = owner * SHP + (src - owner * SH)     # row in gathered table
    q_of_edge = table_row // CH
    rem = (table_row - q_of_edge * CH).astype(np.int64)  # 0..CH-1

    core_of_edge = dst // SH
    local_dst = dst - core_of_edge * SH
    bin_of_edge = local_dst // 128
    off_of_edge = (local_dst % 128).astype(np.float32)

    # per-core grouped edge data
    per_core = []
    counts = np.zeros((C, NB, NQ), dtype=np.int64)
    for c in range(C):
        m = core_of_edge == c
        g = (bin_of_edge[m] * NQ + q_of_edge[m]).astype(np.int64)
        r = rem[m]
        o = off_of_edge[m]
        order = np.lexsort((r, g))
        g, r, o = g[order], r[order], o[order]
        cnt = np.bincount(g, minlength=NB * NQ).reshape(NB, NQ)
        counts[c] = cnt
        per_core.append((g, r, o, cnt))

    nchk = -(-counts.max(axis=0) // 128)             # [NB, NQ] chunklets, >=0
    # ensure no fully empty bin breaks start/stop logic (every bin needs >= 1 chunklet)
    for b in range(NB):
        if nchk[b].sum() == 0:
            nchk[b][0] = 1

    # supergroups: 24 groups of 4 bins + 1 group of 2
    groups = [list(range(i, min(i + SG, NB))) for i in range(0, NB, SG)]

    # slot layout in stream order: for g, for q, for b in g, nchk[b][q] chunklets
    call_info = []      # (g_idx, q, slot_start, n_slots) per gather call
    binq_slot = {}      # (b, q) -> slot_start
    pos = 0
    for gi, g in enumerate(groups):
        for q in range(NQ):
            call_start = pos
            for b in g:
                binq_slot[(b, q)] = pos
                pos += int(nchk[b][q]) * 128
            call_info.append((gi, q, call_start, pos - call_start))
    total_slots = pos
    ncol_chk = total_slots // 128

    # per-core padded idx + dstoff streams
    idx16_all, dstoff_all = [], []
    for c in range(C):
        g_arr, r_arr, o_arr, cnt = per_core[c]
        start_of = np.zeros(NB * NQ + 1, dtype=np.int64)
        np.cumsum(cnt.reshape(-1), out=start_of[1:])
        idx_pad = np.zeros(total_slots, dtype=np.int16)
        off_pad = np.full(total_slots, FILL_OFF, dtype=np.float32)
        for b in range(NB):
            for q in range(NQ):
                n = int(cnt[b][q])
                if n == 0:
                    continue
                s0 = start_of[b * NQ + q]
                d0 = binq_slot[(b, q)]
                idx_pad[d0:d0 + n] = r_arr[s0:s0 + n].astype(np.int16)
                off_pad[d0:d0 + n] = o_arr[s0:s0 + n]
        # wrapped idx per gather call, concatenated along columns
        cols = [
            _wrap_idx(idx_pad[s0:s0 + ns])
            for (_, _, s0, ns) in call_info if ns > 0
        ]
        idx16_all.append(np.concatenate(cols, axis=1))
        dstoff_all.append(
            np.ascontiguousarray(off_pad.reshape(ncol_chk, 128).T))  # [128, ncols]

    # per-core dense inputs
    dstn_t = np.ones((C, 128, NB), dtype=np.float32)
    srcn_t = np.ones((C, 128, NB), dtype=np.float32)
    hs0 = np.zeros((C, SHP, D), dtype=np.float32)
    oacc0 = np.zeros((C, SHP, D), dtype=np.float32)
    for c in range(C):
        sl = slice(c * SH, (c + 1) * SH)
        dn = np.ones(SHP, dtype=np.float32)
        sn = np.ones(SHP, dtype=np.float32)
        dn[:SH] = dst_norm[sl]
        sn[:SH] = src_norm[sl]
        dstn_t[c] = dn.reshape(NB, 128).T
        srcn_t[c] = sn.reshape(NB, 128).T
        f = feats[sl]
        hs0[c, :SH] = f * src_norm[sl][:, None]
        gate = 1.0 / (1.0 + np.exp(-(f @ s)))
        oacc0[c, :SH] = gate[:, None] * f

    sbc = np.tile(s[None, :], (128, 1)).astype(np.float32)
    iota_f = np.tile(np.arange(128, dtype=np.float32)[None, :], (128, 1))

    return dict(
        nchk=nchk, groups=groups, call_info=call_info, binq_slot=binq_slot,
        total_slots=total_slots, ncol_chk=ncol_chk,
        idx16=idx16_all, dstoff=dstoff_all, dstn_t=dstn_t, srcn_t=srcn_t,
        hs0=hs0, oacc0=oacc0, sbc=sbc, iota_f=iota_f,
    )


def build_program(pp, nhops=K):
    import concourse.bass as bass
    import concourse.mybir as mybir
    import concourse.tile as tile
    from concourse import bacc
    from concourse.bass import ds

    F32 = mybir.dt.float32
    I16 = mybir.dt.int16
    nchk = pp["nchk"]
    groups = pp["groups"]
    call_info = pp["call_info"]
    binq_slot = pp["binq_slot"]
    ncol_chk = pp["ncol_chk"]

    # first/last chunklet (q, j) per bin for PSUM start/stop flags
    first_last = {}
    for b in range(NB):
        qs = [q for q in range(NQ) if nchk[b][q] > 0]
        first_last[b] = ((qs[0], 0), (qs[-1], int(nchk[b][qs[-1]]) - 1))

    nc = bacc.Bacc(None)
    feats_in = nc.declare_dram_parameter("hs0_sh", [SHP, D], F32, isOutput=False)
    oacc_in = nc.declare_dram_parameter("oacc0_sh", [SHP, D], F32, isOutput=False)
    idx_in = nc.declare_dram_parameter("idx16", [128, pp["total_slots"] // 16], I16, isOutput=False)
    doff_in = nc.declare_dram_parameter("dstoff", [128, ncol_chk], F32, isOutput=False)
    dstn_in = nc.declare_dram_parameter("dstn_t", [128, NB], F32, isOutput=False)
    srcn_in = nc.declare_dram_parameter("srcn_t", [128, NB], F32, isOutput=False)
    sbc_in = nc.declare_dram_parameter("sbc", [128, D], F32, isOutput=False)
    iota_in = nc.declare_dram_parameter("iota_f", [128, 128], F32, isOutput=False)
    out_d = nc.declare_dram_parameter("out_sh", [SHP, D], F32, isOutput=True)

    hs_next = nc.dram_tensor("hs_next", [SHP, D], F32)
    tables = [
        nc.dram_tensor("tableA", [TBL, D], F32, addr_space="Shared"),
        nc.dram_tensor("tableB", [TBL, D], F32, addr_space="Shared"),
    ]
    rg = [list(range(C))]

    with tile.TileContext(nc) as tc:
        with tc.tile_pool(name="persist", bufs=1) as pers, \
             tc.tile_pool(name="stream", bufs=2) as st, \
             tc.tile_pool(name="psum", bufs=1, space="PSUM") as psp:
            iota = pers.tile([128, 128], F32)
            nc.sync.dma_start(out=iota[:], in_=iota_in[:, :])
            sbc = pers.tile([128, D], F32)
            nc.sync.dma_start(out=sbc[:], in_=sbc_in[:, :])
            dstn = pers.tile([128, NB], F32)
            nc.sync.dma_start(out=dstn[:], in_=dstn_in[:, :])
            srcn = pers.tile([128, NB], F32)
            nc.sync.dma_start(out=srcn[:], in_=srcn_in[:, :])
            oacc = pers.tile([128, NB, D], F32)
            nc.sync.dma_start(
                out=oacc[:], in_=oacc_in[:, :].rearrange("(b p) d -> p b d", p=128))

            # initial table: hs0 -> hs_next (internal) -> AllGather -> tableA
            nc.sync.dma_start(out=hs_next[:, :], in_=feats_in[:, :])
            nc.gpsimd.collective_compute(
                "AllGather", mybir.AluOpType.bypass, replica_groups=rg,
                ins=[hs_next.ap().opt()], outs=[tables[0].ap().opt()])

            for k in range(nhops):
                src_t = tables[k % 2]
                dst_t = tables[1 - k % 2]
                for gi, g in enumerate(groups):
                    psums = {
                        b: psp.tile([128, 128], F32, name=f"ps{gi % 2}_{bl}")
                        for bl, b in enumerate(g)
                    }
                    for q in range(NQ):
                        blk = sum(int(nchk[b][q]) for b in g)
                        if blk == 0:
                            continue
                        s0 = binq_slot[(g[0], q)]
                        idxt = st.tile([128, blk * 8], I16, tag="idx")
                        nc.sync.dma_start(
                            out=idxt[:], in_=idx_in[:, ds(s0 // 16, blk * 8)])
                        doff = st.tile([128, blk], F32, tag="doff")
                        nc.sync.dma_start(
                            out=doff[:], in_=doff_in[:, ds(s0 // 128, blk)])
                        msgs = st.tile([128, blk, 128], F32, tag="msgs")
                        nc.gpsimd.dma_gather(
                            out_ap=msgs[:],
                            in_ap=src_t[q * CH:(q + 1) * CH, :],
                            idxs_ap=idxt[:],
                            num_idxs=blk * 128, num_idxs_reg=blk * 128,
                            elem_size=128, single_packet=False)
                        oh = st.tile([128, blk, 128], F32, tag="oh")
                        nc.vector.tensor_tensor(
                            oh[:],
                            iota[:, None, :].to_broadcast((128, blk, 128)),
                            doff[:, :, None].to_broadcast((128, blk, 128)),
                            mybir.AluOpType.is_equal)
                        lpos = 0
                        for b in g:
                            (fq, fj), (lq, lj) = first_last[b]
                            for j in range(int(nchk[b][q])):
                                nc.tensor.matmul(
                                    psums[b][:], oh[:, lpos, :], msgs[:, lpos, :],
                                    start=(q == fq and j == fj),
                                    stop=(q == lq and j == lj))
                                lpos += 1
                    hsn = st.tile([128, len(g), 128], F32, tag="hsn")
                    for bl, b in enumerate(g):
                        hn = st.tile([128, 128], F32, tag="hn")
                        nc.vector.tensor_tensor(
                            hn[:], psums[b][:],
                            dstn[:, b, None].to_broadcast((128, 128)),
                            mybir.AluOpType.mult)
                        tmp = st.tile([128, 128], F32, tag="tmp")
                        nc.vector.tensor_tensor(
                            tmp[:], hn[:], sbc[:], mybir.AluOpType.mult)
                        gcol = st.tile([128, 1], F32, tag="gcol")
                        nc.vector.tensor_reduce(
                            gcol[:], tmp[:], mybir.AxisListType.X,
                            mybir.AluOpType.add)
                        sig = st.tile([128, 1], F32, tag="sig")
                        nc.scalar.activation(
                            sig[:], gcol[:],
                            mybir.ActivationFunctionType.Sigmoid)
                        contrib = st.tile([128, 128], F32, tag="contrib")
                        nc.vector.tensor_tensor(
                            contrib[:], hn[:],
                            sig[:, :].to_broadcast((128, 128)),
                            mybir.AluOpType.mult)
                        nc.vector.tensor_tensor(
                            oacc[:, b, :], oacc[:, b, :], contrib[:],
                            mybir.AluOpType.add)
                        nc.vector.tensor_tensor(
                            hsn[:, bl, :], hn[:],
                            srcn[:, b, None].to_broadcast((128, 128)),
                            mybir.AluOpType.mult)
                    nc.sync.dma_start(
                        out=hs_next[g[0] * 128:(g[-1] + 1) * 128, :].rearrange(
                            "(b p) d -> p b d", p=128),
                        in_=hsn[:])
                if k < nhops - 1:
                    nc.gpsimd.collective_compute(
                        "AllGather", mybir.AluOpType.bypass, replica_groups=rg,
                        ins=[hs_next.ap().opt()], outs=[dst_t.ap().opt()])
            nc.sync.dma_start(
                out=out_d[:, :].rearrange("(b p) d -> p b d", p=128),
                in_=oacc[:])
    nc.finalize()
    return nc


def kernel(feats, s, src, dst, nhops=K, want_profile=False, profile_dir=None):
    from concourse.bass_utils import run_bass_kernel_spmd

    pp = preprocess(feats, s, src, dst)
    nc = build_program(pp, nhops=nhops)

    in_maps = []
    for c in range(C):
        in_maps.append({
            "hs0_sh": pp["hs0"][c],
            "oacc0_sh": pp["oacc0"][c],
            "idx16": pp["idx16"][c],
            "dstoff": pp["dstoff"][c],
            "dstn_t": pp["dstn_t"][c],
            "srcn_t": pp["srcn_t"][c],
            "sbc": pp["sbc"],
            "iota_f": pp["iota_f"],
        })

    if want_profile:
        import contextlib
        from trn_agent_boot.trn_boot import _ntff_profile_via_ctypes
        from concourse import bass2jax
        hook = _ntff_profile_via_ctypes('/opt/axon/libaxon_pjrt.so')
        ctx = hook(profile_dir, [0]) if hook else contextlib.nullcontext()
        with ctx:
            results = bass2jax.run_bass_via_pjrt(nc, in_maps, n_cores=C)
    else:
        results = run_bass_kernel_spmd(nc, in_maps, list(range(C))).results

    out = np.empty((N, D), dtype=np.float32)
    for c in range(C):
        out[c * SH:(c + 1) * SH] = results[c]["out_sh"][:SH]
    return out



# revision 7
# speedup vs baseline: 2.2973x; 2.2973x over previous
"""DAGNN propagation kernel for 8 Trainium2 NeuronCores (v2).

Strategy: partition nodes across the 8 cores (12,500 nodes each). Each hop is
an SpMM h_next = A_norm @ h computed per core for its own nodes:
  - the scaled feature table (h * src_norm, bf16) is replicated to every core
    by four quarter-AllGathers issued mid-hop (overlapped with compute)
  - per edge, the source row is fetched with dma_gather (256B bf16 rows).
    Gather descriptor generation on the Q7 cores is the machine bottleneck
    (~9ns/row on one queue pair), so gather calls are striped across the
    4 SWDGE queues: queue q runs on Q7 core pair (2q, 2q+1), and the Pool
    engine's 4-deep exec queue lets all four generate concurrently.
  - the scatter-add over destinations is a sequence of one-hot bf16 matmuls
    accumulating into a PSUM bank per 4-bin supergroup; one-hot matrices are
    built on-chip by comparing an iota row against the edge's destination
    offset
  - the DAGNN gate (sigmoid(h_k . s) * h_k) is accumulated per supergroup
    right after each hop so hop results never round-trip through HBM
Host-side numpy does the graph preprocessing once: degree norms, edge
partitioning by (destination bin, source table quarter), padding to 128-edge
chunklets uniform across cores (SPMD requires one program for all cores).
"""

import sys
import numpy as np

for _p in ("/root/.axon_site", "/root/.axon_site/_ro/trn_rl_repo", "/opt/trn_rl_repo"):
    if _p not in sys.path:
        sys.path.append(_p)

import ml_dtypes

BF16NP = np.dtype(ml_dtypes.bfloat16)

N = 100000
D = 128
K = 10
C = 8
SH = 12500          # nodes per core
NB = 98             # destination bins per core
SHP = NB * 128      # padded shard rows = 12544
NQ = 4
SG = 4              # bins per supergroup
QBINS = [24, 24, 24, 26]          # bins per table quarter
QSTART = [0, 24, 48, 72]
ROWSQ = [b * 128 for b in QBINS]  # per-core rows per quarter
TROWS = [C * r for r in ROWSQ]    # gathered quarter-table rows (max 26624 < int16)
FILL_OFF = 200.0    # dstoff sentinel for padding edges -> all-zero one-hot row
SINGLE_PACKET = False
QUEUE_STRIPE = True


def _wrap_idx(arr):
    """int16 index array [n] (n % 128 == 0) -> dma_gather wrapped layout
    [128, n//16]: logical i at [i % 16, i // 16], replicated to all 8 Q7
    cores (partitions 16r + i%16)."""
    n = arr.shape[0]
    w = arr.reshape(n // 16, 16).T  # [16, n//16]
    return np.tile(w, (8, 1)).astype(np.int16)


def preprocess(feats, s, src, dst):
    feats = np.asarray(feats, dtype=np.float32)
    s = np.asarray(s, dtype=np.float32).reshape(D)
    src = np.asarray(src, dtype=np.int64)
    dst = np.asarray(dst, dtype=np.int64)

    in_deg = np.bincount(dst, minlength=N).astype(np.float32)
    out_deg = np.bincount(src, minlength=N).astype(np.float32)
    dst_norm = np.clip(in_deg, 1.0, None) ** -0.5
    src_norm = np.clip(out_deg, 1.0, None) ** -0.5

    rowsq_a = np.array(ROWSQ, dtype=np.int64)
    qstart_a = np.array(QSTART, dtype=np.int64)

    owner = src // SH
    i_loc = src - owner * SH
    bs = i_loc >> 7
    q_of_edge = np.searchsorted(np.array([24, 48, 72]), bs, side="right")
    rem = owner * rowsq_a[q_of_edge] + (i_loc - 128 * qstart_a[q_of_edge])

    core_of_edge = dst // SH
    local_dst = dst - core_of_edge * SH
    bin_of_edge = local_dst >> 7
    off_of_edge = (local_dst & 127).astype(np.float32)

    per_core = []
    counts = np.zeros((C, NB, NQ), dtype=np.int64)
    for c in range(C):
        m = core_of_edge == c
        g = (bin_of_edge[m] * NQ + q_of_edge[m]).astype(np.int64)
        r = rem[m]
        o = off_of_edge[m]
        order = np.lexsort((r, g))
        g, r, o = g[order], r[order], o[order]
        cnt = np.bincount(g, minlength=NB * NQ).reshape(NB, NQ)
        counts[c] = cnt
        per_core.append((g, r, o, cnt))

    nchk = -(-counts.max(axis=0) // 128)             # [NB, NQ] chunklets
    for b in range(NB):
        if nchk[b].sum() == 0:
            nchk[b][0] = 1

    groups = [list(range(i, min(i + SG, NB))) for i in range(0, NB, SG)]

    # slot layout in stream order: for g, for q, for b in g, nchk[b][q] chunklets
    call_info = []      # (g_idx, q, slot_start, n_slots) per gather call
    binq_slot = {}
    pos = 0
    for gi, g in enumerate(groups):
        for q in range(NQ):
            call_start = pos
            for b in g:
                binq_slot[(b, q)] = pos
                pos += int(nchk[b][q]) * 128
            call_info.append((gi, q, call_start, pos - call_start))
    total_slots = pos
    ncol_chk = total_slots // 128

    idx16_all, dstoff_all = [], []
    for c in range(C):
        g_arr, r_arr, o_arr, cnt = per_core[c]
        start_of = np.zeros(NB * NQ + 1, dtype=np.int64)
        np.cumsum(cnt.reshape(-1), out=start_of[1:])
        idx_pad = np.zeros(total_slots, dtype=np.int16)
        off_pad = np.full(total_slots, FILL_OFF, dtype=np.float32)
        for b in range(NB):
            for q in range(NQ):
                n = int(cnt[b][q])
                if n == 0:
                    continue
                s0 = start_of[b * NQ + q]
                d0 = binq_slot[(b, q)]
                idx_pad[d0:d0 + n] = r_arr[s0:s0 + n].astype(np.int16)
                off_pad[d0:d0 + n] = o_arr[s0:s0 + n]
        cols = [
            _wrap_idx(idx_pad[s0:s0 + ns])
            for (_, _, s0, ns) in call_info if ns > 0
        ]
        idx16_all.append(np.concatenate(cols, axis=1))
        dstoff_all.append(np.ascontiguousarray(
            off_pad.reshape(ncol_chk, 128).T).astype(BF16NP))  # [128, ncols]

    # per-core dense inputs
    dstn_t = np.ones((C, 128, NB), dtype=np.float32)
    srcn_t = np.ones((C, 128, NB), dtype=np.float32)
    oacc0 = np.zeros((C, SHP, D), dtype=np.float32)
    hs0q = [[None] * NQ for _ in range(C)]
    for c in range(C):
        sl = slice(c * SH, (c + 1) * SH)
        dn = np.ones(SHP, dtype=np.float32)
        sn = np.ones(SHP, dtype=np.float32)
        dn[:SH] = dst_norm[sl]
        sn[:SH] = src_norm[sl]
        dstn_t[c] = dn.reshape(NB, 128).T
        srcn_t[c] = sn.reshape(NB, 128).T
        f = feats[sl]
        hs_full = np.zeros((SHP, D), dtype=np.float32)
        hs_full[:SH] = f * src_norm[sl][:, None]
        for q in range(NQ):
            r0 = 128 * QSTART[q]
            hs0q[c][q] = hs_full[r0:r0 + ROWSQ[q]].astype(BF16NP)
        gate = 1.0 / (1.0 + np.exp(-(f @ s)))
        oacc0[c, :SH] = gate[:, None] * f

    sbc = np.tile(s[None, :], (128, 1)).astype(np.float32)
    iota_f = np.tile(np.arange(128)[None, :], (128, 1)).astype(BF16NP)

    return dict(
        nchk=nchk, groups=groups, call_info=call_info, binq_slot=binq_slot,
        total_slots=total_slots, ncol_chk=ncol_chk,
        idx16=idx16_all, dstoff=dstoff_all, dstn_t=dstn_t, srcn_t=srcn_t,
        hs0q=hs0q, oacc0=oacc0, sbc=sbc, iota_f=iota_f,
    )


def build_program(pp, nhops=K):
    import concourse.bass as bass
    import concourse.mybir as mybir
    import concourse.tile as tile
    from concourse import bacc
    from concourse.bass import ds

    F32 = mybir.dt.float32
    BF = mybir.dt.bfloat16
    I16 = mybir.dt.int16
    nchk = pp["nchk"]
    groups = pp["groups"]
    binq_slot = pp["binq_slot"]
    ncol_chk = pp["ncol_chk"]

    # first/last chunklet (q, j) per bin for PSUM start/stop flags
    first_last = {}
    for b in range(NB):
        qs = [q for q in range(NQ) if nchk[b][q] > 0]
        first_last[b] = ((qs[0], 0), (qs[-1], int(nchk[b][qs[-1]]) - 1))

    # AllGather issue point: after this group index, quarter q's bins are done
    ag_after = {5: 0, 11: 1, 17: 2, len(groups) - 1: 3}

    nc = bacc.Bacc(None, num_swdge_queues=NQ if QUEUE_STRIPE else 1)
    oacc_in = nc.declare_dram_parameter("oacc0_sh", [SHP, D], F32, isOutput=False)
    idx_in = nc.declare_dram_parameter("idx16", [128, pp["total_slots"] // 16], I16, isOutput=False)
    doff_in = nc.declare_dram_parameter("dstoff", [128, ncol_chk], BF, isOutput=False)
    dstn_in = nc.declare_dram_parameter("dstn_t", [128, NB], F32, isOutput=False)
    srcn_in = nc.declare_dram_parameter("srcn_t", [128, NB], F32, isOutput=False)
    sbc_in = nc.declare_dram_parameter("sbc", [128, D], F32, isOutput=False)
    iota_in = nc.declare_dram_parameter("iota_f", [128, 128], BF, isOutput=False)
    hs0_in = [
        nc.declare_dram_parameter(f"hs0_q{q}", [ROWSQ[q], D], BF, isOutput=False)
        for q in range(NQ)
    ]
    out_d = nc.declare_dram_parameter("out_sh", [SHP, D], F32, isOutput=True)

    stages = [[nc.dram_tensor(f"hstg{k}_{q}", [ROWSQ[q], D], BF) for q in range(NQ)]
              for k in range(nhops)]
    tabs = [[nc.dram_tensor(f"tab{k}_{q}", [TROWS[q], D], BF, addr_space="Shared")
             for q in range(NQ)] for k in range(nhops)]
    rg = [list(range(C))]

    with tile.TileContext(nc) as tc:
        with tc.tile_pool(name="persist", bufs=1) as pers, \
             tc.tile_pool(name="stream", bufs=2) as st, \
             tc.tile_pool(name="msgs", bufs=6) as msp, \
             tc.tile_pool(name="psum", bufs=1, space="PSUM") as psp:
            iota = pers.tile([128, 128], BF)
            nc.sync.dma_start(out=iota[:], in_=iota_in[:, :])
            sbc = pers.tile([128, D], F32)
            nc.sync.dma_start(out=sbc[:], in_=sbc_in[:, :])
            dstn = pers.tile([128, NB], F32)
            nc.sync.dma_start(out=dstn[:], in_=dstn_in[:, :])
            srcn = pers.tile([128, NB], F32)
            nc.sync.dma_start(out=srcn[:], in_=srcn_in[:, :])
            oacc = pers.tile([128, NB, D], F32)
            nc.sync.dma_start(
                out=oacc[:], in_=oacc_in[:, :].rearrange("(b p) d -> p b d", p=128))
            hsq = pers.tile([128, NB, D], BF)

            # initial table: hs0 quarters -> internal stage -> AllGather
            for q in range(NQ):
                nc.sync.dma_start(out=stages[0][q][:, :], in_=hs0_in[q][:, :])
                nc.gpsimd.collective_compute(
                    "AllGather", mybir.AluOpType.bypass, replica_groups=rg,
                    ins=[stages[0][q].ap().opt()], outs=[tabs[0][q].ap().opt()])

            for k in range(nhops):
                for gi, g in enumerate(groups):
                    ng = len(g)
                    # one PSUM bank per bin: a start=True matmul clears
                    # has_written for its whole bank, so bins cannot share one
                    psums = [psp.tile([128, 128], F32, name=f"ps{gi % 2}_{bl}")
                             for bl in range(ng)]
                    for q in range(NQ):
                        blk = sum(int(nchk[b][q]) for b in g)
                        if blk == 0:
                            continue
                        s0 = binq_slot[(g[0], q)]
                        idxt = st.tile([128, blk * 8], I16, tag=f"idx{q}")
                        nc.sync.dma_start(
                            out=idxt[:], in_=idx_in[:, ds(s0 // 16, blk * 8)])
                        doff = st.tile([128, blk], BF, tag=f"doff{q}")
                        nc.sync.dma_start(
                            out=doff[:], in_=doff_in[:, ds(s0 // 128, blk)])
                        msgs = msp.tile([128, blk, 128], BF, tag="msgs")
                        nc.gpsimd.dma_gather(
                            out_ap=msgs[:],
                            in_ap=tabs[k][q][:, :],
                            idxs_ap=idxt[:],
                            num_idxs=blk * 128, num_idxs_reg=blk * 128,
                            elem_size=128, single_packet=SINGLE_PACKET,
                            queue_num=q if QUEUE_STRIPE else 0)
                        oh = st.tile([128, blk, 128], BF, tag="oh")
                        nc.vector.tensor_tensor(
                            oh[:],
                            iota[:, None, :].to_broadcast((128, blk, 128)),
                            doff[:, :, None].to_broadcast((128, blk, 128)),
                            mybir.AluOpType.is_equal)
                        lpos = 0
                        for bl, b in enumerate(g):
                            (fq, fj), (lq, lj) = first_last[b]
                            for j in range(int(nchk[b][q])):
                                nc.tensor.matmul(
                                    psums[bl][:], oh[:, lpos, :], msgs[:, lpos, :],
                                    start=(q == fq and j == fj),
                                    stop=(q == lq and j == lj))
                                lpos += 1
                    # postprocess the supergroup: norms, gate, readout accum
                    g0 = g[0]
                    hn = st.tile([128, ng, 128], F32, tag="hn")
                    for bl in range(ng):
                        nc.vector.tensor_tensor(
                            hn[:, bl, :], psums[bl][:],
                            dstn[:, g0 + bl, None].to_broadcast((128, 128)),
                            mybir.AluOpType.mult)
                    tmp = st.tile([128, ng, 128], F32, tag="tmp")
                    nc.vector.tensor_tensor(
                        tmp[:], hn[:],
                        sbc[:, None, :].to_broadcast((128, ng, 128)),
                        mybir.AluOpType.mult)
                    gcol = st.tile([128, ng, 1], F32, tag="gcol")
                    nc.vector.tensor_reduce(
                        gcol[:], tmp[:], mybir.AxisListType.X,
                        mybir.AluOpType.add)
                    sig = st.tile([128, ng, 1], F32, tag="sig")
                    nc.scalar.activation(
                        sig[:], gcol[:], mybir.ActivationFunctionType.Sigmoid)
                    contrib = st.tile([128, ng, 128], F32, tag="contrib")
                    nc.vector.tensor_tensor(
                        contrib[:], hn[:],
                        sig[:, :, :].to_broadcast((128, ng, 128)),
                        mybir.AluOpType.mult)
                    nc.vector.tensor_tensor(
                        oacc[:, g0:g0 + ng, :], oacc[:, g0:g0 + ng, :],
                        contrib[:], mybir.AluOpType.add)
                    if k < nhops - 1:
                        nc.vector.tensor_tensor(
                            hsq[:, g0:g0 + ng, :], hn[:],
                            srcn[:, g0:g0 + ng, None].to_broadcast((128, ng, 128)),
                            mybir.AluOpType.mult)
                        if gi in ag_after:
                            q = ag_after[gi]
                            nc.sync.dma_start(
                                out=stages[k + 1][q][:, :].rearrange(
                                    "(b p) d -> p b d", p=128),
                                in_=hsq[:, QSTART[q]:QSTART[q] + QBINS[q], :])
                            nc.gpsimd.collective_compute(
                                "AllGather", mybir.AluOpType.bypass,
                                replica_groups=rg,
                                ins=[stages[k + 1][q].ap().opt()],
                                outs=[tabs[k + 1][q].ap().opt()])
            nc.sync.dma_start(
                out=out_d[:, :].rearrange("(b p) d -> p b d", p=128),
                in_=oacc[:])
    nc.finalize()
    return nc


def kernel(feats, s, src, dst, nhops=K, want_profile=False, profile_dir=None):
    from concourse.bass_utils import run_bass_kernel_spmd

    pp = preprocess(feats, s, src, dst)
    nc = build_program(pp, nhops=nhops)

    in_maps = []
    for c in range(C):
        m = {
            "oacc0_sh": pp["oacc0"][c],
            "idx16": pp["idx16"][c],
            "dstoff": pp["dstoff"][c],
            "dstn_t": pp["dstn_t"][c],
            "srcn_t": pp["srcn_t"][c],
            "sbc": pp["sbc"],
            "iota_f": pp["iota_f"],
        }
        for q in range(NQ):
            m[f"hs0_q{q}"] = pp["hs0q"][c][q]
        in_maps.append(m)

    if want_profile:
        import contextlib
        from trn_agent_boot.trn_boot import _ntff_profile_via_ctypes
        from concourse import bass2jax
        hook = _ntff_profile_via_ctypes('/opt/axon/libaxon_pjrt.so')
        ctx = hook(profile_dir, [0]) if hook else contextlib.nullcontext()
        with ctx:
            results = bass2jax.run_bass_via_pjrt(nc, in_maps, n_cores=C)
    else:
        results = run_bass_kernel_spmd(nc, in_maps, list(range(C))).results

    out = np.empty((N, D), dtype=np.float32)
    for c in range(C):
        out[c * SH:(c + 1) * SH] = results[c]["out_sh"][:SH]
    return out


# revision 9
# speedup vs baseline: 2.3153x; 1.0079x over previous
"""DAGNN propagation kernel for 8 Trainium2 NeuronCores (v2).

Strategy: partition nodes across the 8 cores (12,500 nodes each). Each hop is
an SpMM h_next = A_norm @ h computed per core for its own nodes:
  - the scaled feature table (h * src_norm, bf16) is replicated to every core
    by four quarter-AllGathers issued mid-hop (overlapped with compute)
  - per edge, the source row is fetched with dma_gather (256B bf16 rows).
    Gather descriptor generation on the Q7 cores is the machine bottleneck
    (~9ns/row on one queue pair), so gather calls are striped across the
    4 SWDGE queues: queue q runs on Q7 core pair (2q, 2q+1), and the Pool
    engine's 4-deep exec queue lets all four generate concurrently.
  - the scatter-add over destinations is a sequence of one-hot bf16 matmuls
    accumulating into a PSUM bank per 4-bin supergroup; one-hot matrices are
    built on-chip by comparing an iota row against the edge's destination
    offset
  - the DAGNN gate (sigmoid(h_k . s) * h_k) is accumulated per supergroup
    right after each hop so hop results never round-trip through HBM
Host-side numpy does the graph preprocessing once: degree norms, edge
partitioning by (destination bin, source table quarter), padding to 128-edge
chunklets uniform across cores (SPMD requires one program for all cores).
"""

import sys
import numpy as np

for _p in ("/root/.axon_site", "/root/.axon_site/_ro/trn_rl_repo", "/opt/trn_rl_repo"):
    if _p not in sys.path:
        sys.path.append(_p)

import ml_dtypes

BF16NP = np.dtype(ml_dtypes.bfloat16)

N = 100000
D = 128
K = 10
C = 8
SH = 12500          # nodes per core
NB = 98             # destination bins per core
SHP = NB * 128      # padded shard rows = 12544
NQ = 4
SG = 4              # bins per supergroup
QBINS = [24, 24, 24, 26]          # bins per table quarter
QSTART = [0, 24, 48, 72]
ROWSQ = [b * 128 for b in QBINS]  # per-core rows per quarter
TROWS = [C * r for r in ROWSQ]    # gathered quarter-table rows (max 26624 < int16)
FILL_OFF = 200.0    # dstoff sentinel for padding edges -> all-zero one-hot row
SINGLE_PACKET = False
QUEUE_STRIPE = True


def _wrap_idx(arr):
    """int16 index array [n] (n % 128 == 0) -> dma_gather wrapped layout
    [128, n//16]: logical i at [i % 16, i // 16], replicated to all 8 Q7
    cores (partitions 16r + i%16)."""
    n = arr.shape[0]
    w = arr.reshape(n // 16, 16).T  # [16, n//16]
    return np.tile(w, (8, 1)).astype(np.int16)


def preprocess(feats, s, src, dst):
    feats = np.asarray(feats, dtype=np.float32)
    s = np.asarray(s, dtype=np.float32).reshape(D)
    src = np.asarray(src, dtype=np.int64)
    dst = np.asarray(dst, dtype=np.int64)

    in_deg = np.bincount(dst, minlength=N).astype(np.float32)
    out_deg = np.bincount(src, minlength=N).astype(np.float32)
    dst_norm = np.clip(in_deg, 1.0, None) ** -0.5
    src_norm = np.clip(out_deg, 1.0, None) ** -0.5

    rowsq_a = np.array(ROWSQ, dtype=np.int64)
    qstart_a = np.array(QSTART, dtype=np.int64)

    owner = src // SH
    i_loc = src - owner * SH
    bs = i_loc >> 7
    q_of_edge = np.searchsorted(np.array([24, 48, 72]), bs, side="right")
    rem = owner * rowsq_a[q_of_edge] + (i_loc - 128 * qstart_a[q_of_edge])

    core_of_edge = dst // SH
    local_dst = dst - core_of_edge * SH
    bin_of_edge = local_dst >> 7
    off_of_edge = (local_dst & 127).astype(np.float32)

    per_core = []
    counts = np.zeros((C, NB, NQ), dtype=np.int64)
    for c in range(C):
        m = core_of_edge == c
        g = (bin_of_edge[m] * NQ + q_of_edge[m]).astype(np.int64)
        r = rem[m]
        o = off_of_edge[m]
        order = np.lexsort((r, g))
        g, r, o = g[order], r[order], o[order]
        cnt = np.bincount(g, minlength=NB * NQ).reshape(NB, NQ)
        counts[c] = cnt
        per_core.append((g, r, o, cnt))

    nchk = -(-counts.max(axis=0) // 128)             # [NB, NQ] chunklets
    for b in range(NB):
        if nchk[b].sum() == 0:
            nchk[b][0] = 1

    groups = [list(range(i, min(i + SG, NB))) for i in range(0, NB, SG)]

    # slot layout in stream order: for g, for q, for b in g, nchk[b][q] chunklets
    call_info = []      # (g_idx, q, slot_start, n_slots) per gather call
    binq_slot = {}
    pos = 0
    for gi, g in enumerate(groups):
        for q in range(NQ):
            call_start = pos
            for b in g:
                binq_slot[(b, q)] = pos
                pos += int(nchk[b][q]) * 128
            call_info.append((gi, q, call_start, pos - call_start))
    total_slots = pos
    ncol_chk = total_slots // 128

    idx16_all, dstoff_all = [], []
    for c in range(C):
        g_arr, r_arr, o_arr, cnt = per_core[c]
        start_of = np.zeros(NB * NQ + 1, dtype=np.int64)
        np.cumsum(cnt.reshape(-1), out=start_of[1:])
        idx_pad = np.zeros(total_slots, dtype=np.int16)
        off_pad = np.full(total_slots, FILL_OFF, dtype=np.float32)
        for b in range(NB):
            for q in range(NQ):
                n = int(cnt[b][q])
                if n == 0:
                    continue
                s0 = start_of[b * NQ + q]
                d0 = binq_slot[(b, q)]
                idx_pad[d0:d0 + n] = r_arr[s0:s0 + n].astype(np.int16)
                off_pad[d0:d0 + n] = o_arr[s0:s0 + n]
        cols = [
            _wrap_idx(idx_pad[s0:s0 + ns])
            for (_, _, s0, ns) in call_info if ns > 0
        ]
        idx16_all.append(np.concatenate(cols, axis=1))
        dstoff_all.append(np.ascontiguousarray(
            off_pad.reshape(ncol_chk, 128).T).astype(BF16NP))  # [128, ncols]

    # per-core dense inputs
    dstn_t = np.ones((C, 128, NB), dtype=np.float32)
    srcn_t = np.ones((C, 128, NB), dtype=np.float32)
    oacc0 = np.zeros((C, SHP, D), dtype=np.float32)
    hs0q = [[None] * NQ for _ in range(C)]
    for c in range(C):
        sl = slice(c * SH, (c + 1) * SH)
        dn = np.ones(SHP, dtype=np.float32)
        sn = np.ones(SHP, dtype=np.float32)
        dn[:SH] = dst_norm[sl]
        sn[:SH] = src_norm[sl]
        dstn_t[c] = dn.reshape(NB, 128).T
        srcn_t[c] = sn.reshape(NB, 128).T
        f = feats[sl]
        hs_full = np.zeros((SHP, D), dtype=np.float32)
        hs_full[:SH] = f * src_norm[sl][:, None]
        for q in range(NQ):
            r0 = 128 * QSTART[q]
            hs0q[c][q] = hs_full[r0:r0 + ROWSQ[q]].astype(BF16NP)
        gate = 1.0 / (1.0 + np.exp(-(f @ s)))
        oacc0[c, :SH] = gate[:, None] * f

    sbc = np.tile(s[None, :], (128, 1)).astype(np.float32)
    iota_f = np.tile(np.arange(128)[None, :], (128, 1)).astype(BF16NP)

    return dict(
        nchk=nchk, groups=groups, call_info=call_info, binq_slot=binq_slot,
        total_slots=total_slots, ncol_chk=ncol_chk,
        idx16=idx16_all, dstoff=dstoff_all, dstn_t=dstn_t, srcn_t=srcn_t,
        hs0q=hs0q, oacc0=oacc0, sbc=sbc, iota_f=iota_f,
    )


def build_program(pp, nhops=K):
    import concourse.bass as bass
    import concourse.mybir as mybir
    import concourse.tile as tile
    from concourse import bacc
    from concourse.bass import ds

    F32 = mybir.dt.float32
    BF = mybir.dt.bfloat16
    I16 = mybir.dt.int16
    nchk = pp["nchk"]
    groups = pp["groups"]
    binq_slot = pp["binq_slot"]
    ncol_chk = pp["ncol_chk"]

    # first/last chunklet (q, j) per bin for PSUM start/stop flags
    first_last = {}
    for b in range(NB):
        qs = [q for q in range(NQ) if nchk[b][q] > 0]
        first_last[b] = ((qs[0], 0), (qs[-1], int(nchk[b][qs[-1]]) - 1))

    # AllGather issue point: after this group index, quarter q's bins are done
    ag_after = {5: 0, 11: 1, 17: 2, len(groups) - 1: 3}

    nc = bacc.Bacc(None, num_swdge_queues=NQ if QUEUE_STRIPE else 1)
    oacc_in = nc.declare_dram_parameter("oacc0_sh", [SHP, D], F32, isOutput=False)
    idx_in = nc.declare_dram_parameter("idx16", [128, pp["total_slots"] // 16], I16, isOutput=False)
    doff_in = nc.declare_dram_parameter("dstoff", [128, ncol_chk], BF, isOutput=False)
    dstn_in = nc.declare_dram_parameter("dstn_t", [128, NB], F32, isOutput=False)
    srcn_in = nc.declare_dram_parameter("srcn_t", [128, NB], F32, isOutput=False)
    sbc_in = nc.declare_dram_parameter("sbc", [128, D], F32, isOutput=False)
    iota_in = nc.declare_dram_parameter("iota_f", [128, 128], BF, isOutput=False)
    hs0_in = [
        nc.declare_dram_parameter(f"hs0_q{q}", [ROWSQ[q], D], BF, isOutput=False)
        for q in range(NQ)
    ]
    out_d = nc.declare_dram_parameter("out_sh", [SHP, D], F32, isOutput=True)

    stages = [[nc.dram_tensor(f"hstg{k}_{q}", [ROWSQ[q], D], BF) for q in range(NQ)]
              for k in range(nhops)]
    tabs = [[nc.dram_tensor(f"tab{k}_{q}", [TROWS[q], D], BF, addr_space="Shared")
             for q in range(NQ)] for k in range(nhops)]
    rg = [list(range(C))]

    with tile.TileContext(nc) as tc:
        with tc.tile_pool(name="persist", bufs=1) as pers, \
             tc.tile_pool(name="stream", bufs=2) as st, \
             tc.tile_pool(name="msgs", bufs=6) as msp, \
             tc.tile_pool(name="psum", bufs=1, space="PSUM") as psp:
            iota = pers.tile([128, 128], BF)
            nc.sync.dma_start(out=iota[:], in_=iota_in[:, :])
            sbc = pers.tile([128, D], F32)
            nc.sync.dma_start(out=sbc[:], in_=sbc_in[:, :])
            dstn = pers.tile([128, NB], F32)
            nc.sync.dma_start(out=dstn[:], in_=dstn_in[:, :])
            srcn = pers.tile([128, NB], F32)
            nc.sync.dma_start(out=srcn[:], in_=srcn_in[:, :])
            oacc = pers.tile([128, NB, D], F32)
            nc.sync.dma_start(
                out=oacc[:], in_=oacc_in[:, :].rearrange("(b p) d -> p b d", p=128))
            hsq = pers.tile([128, NB, D], BF)

            # initial table: hs0 quarters -> internal stage -> AllGather
            for q in range(NQ):
                nc.sync.dma_start(out=stages[0][q][:, :], in_=hs0_in[q][:, :])
                nc.gpsimd.collective_compute(
                    "AllGather", mybir.AluOpType.bypass, replica_groups=rg,
                    ins=[stages[0][q].ap().opt()], outs=[tabs[0][q].ap().opt()])

            for k in range(nhops):
                for gi, g in enumerate(groups):
                    ng = len(g)
                    # one PSUM bank per bin: a start=True matmul clears
                    # has_written for its whole bank, so bins cannot share one
                    psums = [psp.tile([128, 128], F32, name=f"ps{gi % 2}_{bl}")
                             for bl in range(ng)]
                    for q in range(NQ):
                        blk = sum(int(nchk[b][q]) for b in g)
                        if blk == 0:
                            continue
                        s0 = binq_slot[(g[0], q)]
                        idxt = st.tile([128, blk * 8], I16, tag=f"idx{q}")
                        nc.sync.dma_start(
                            out=idxt[:], in_=idx_in[:, ds(s0 // 16, blk * 8)])
                        doff = st.tile([128, blk], BF, tag=f"doff{q}")
                        nc.sync.dma_start(
                            out=doff[:], in_=doff_in[:, ds(s0 // 128, blk)])
                        msgs = msp.tile([128, blk, 128], BF, tag="msgs")
                        nc.gpsimd.dma_gather(
                            out_ap=msgs[:],
                            in_ap=tabs[k][q][:, :],
                            idxs_ap=idxt[:],
                            num_idxs=blk * 128, num_idxs_reg=blk * 128,
                            elem_size=128, single_packet=SINGLE_PACKET,
                            queue_num=q if QUEUE_STRIPE else 0)
                        oh = st.tile([128, blk, 128], BF, tag="oh")
                        nc.vector.tensor_tensor(
                            oh[:],
                            iota[:, None, :].to_broadcast((128, blk, 128)),
                            doff[:, :, None].to_broadcast((128, blk, 128)),
                            mybir.AluOpType.is_equal)
                        lpos = 0
                        for bl, b in enumerate(g):
                            (fq, fj), (lq, lj) = first_last[b]
                            for j in range(int(nchk[b][q])):
                                nc.tensor.matmul(
                                    psums[bl][:], oh[:, lpos, :], msgs[:, lpos, :],
                                    start=(q == fq and j == fj),
                                    stop=(q == lq and j == lj))
                                lpos += 1
                    # postprocess the supergroup: norms, gate, readout accum
                    g0 = g[0]
                    hn = st.tile([128, ng, 128], F32, tag="hn")
                    for bl in range(ng):
                        nc.vector.tensor_tensor(
                            hn[:, bl, :], psums[bl][:],
                            dstn[:, g0 + bl, None].to_broadcast((128, 128)),
                            mybir.AluOpType.mult)
                    tmp = st.tile([128, ng, 128], F32, tag="tmp")
                    nc.vector.tensor_tensor(
                        tmp[:], hn[:],
                        sbc[:, None, :].to_broadcast((128, ng, 128)),
                        mybir.AluOpType.mult)
                    gcol = st.tile([128, ng, 1], F32, tag="gcol")
                    nc.vector.tensor_reduce(
                        gcol[:], tmp[:], mybir.AxisListType.X,
                        mybir.AluOpType.add)
                    sig = st.tile([128, ng, 1], F32, tag="sig")
                    nc.scalar.activation(
                        sig[:], gcol[:], mybir.ActivationFunctionType.Sigmoid)
                    contrib = st.tile([128, ng, 128], F32, tag="contrib")
                    nc.vector.tensor_tensor(
                        contrib[:], hn[:],
                        sig[:, :, :].to_broadcast((128, ng, 128)),
                        mybir.AluOpType.mult)
                    nc.vector.tensor_tensor(
                        oacc[:, g0:g0 + ng, :], oacc[:, g0:g0 + ng, :],
                        contrib[:], mybir.AluOpType.add)
                    if k < nhops - 1:
                        nc.vector.tensor_tensor(
                            hsq[:, g0:g0 + ng, :], hn[:],
                            srcn[:, g0:g0 + ng, None].to_broadcast((128, ng, 128)),
                            mybir.AluOpType.mult)
                        if gi in ag_after:
                            q = ag_after[gi]
                            nc.sync.dma_start(
                                out=stages[k + 1][q][:, :].rearrange(
                                    "(b p) d -> p b d", p=128),
                                in_=hsq[:, QSTART[q]:QSTART[q] + QBINS[q], :])
                            nc.gpsimd.collective_compute(
                                "AllGather", mybir.AluOpType.bypass,
                                replica_groups=rg,
                                ins=[stages[k + 1][q].ap().opt()],
                                outs=[tabs[k + 1][q].ap().opt()])
            nc.sync.dma_start(
                out=out_d[:, :].rearrange("(b p) d -> p b d", p=128),
                in_=oacc[:])
    nc.finalize()
    return nc


def kernel(feats, s, src, dst, nhops=K, want_profile=False, profile_dir=None):
    from concourse.bass_utils import run_bass_kernel_spmd

    pp = preprocess(feats, s, src, dst)
    nc = build_program(pp, nhops=nhops)

    in_maps = []
    for c in range(C):
        m = {
            "oacc0_sh": pp["oacc0"][c],
            "idx16": pp["idx16"][c],
            "dstoff": pp["dstoff"][c],
            "dstn_t": pp["dstn_t"][c],
            "srcn_t": pp["srcn_t"][c],
            "sbc": pp["sbc"],
            "iota_f": pp["iota_f"],
        }
        for q in range(NQ):
            m[f"hs0_q{q}"] = pp["hs0q"][c][q]
        in_maps.append(m)

    if want_profile:
        import contextlib
        from trn_agent_boot.trn_boot import _ntff_profile_via_ctypes
        from concourse import bass2jax
        hook = _ntff_profile_via_ctypes('/opt/axon/libaxon_pjrt.so')
        ctx = hook(profile_dir, [0]) if hook else contextlib.nullcontext()
        with ctx:
            results = bass2jax.run_bass_via_pjrt(nc, in_maps, n_cores=C)
    else:
        results = run_bass_kernel_spmd(nc, in_maps, list(range(C))).results

    out = np.empty((N, D), dtype=np.float32)
    for c in range(C):
        out[c * SH:(c + 1) * SH] = results[c]["out_sh"][:SH]
    return out


# revision 14
# speedup vs baseline: 2.4799x; 1.0711x over previous
"""DAGNN propagation kernel for 8 Trainium2 NeuronCores (v2).

Strategy: partition nodes across the 8 cores (12,500 nodes each). Each hop is
an SpMM h_next = A_norm @ h computed per core for its own nodes:
  - the scaled feature table (h * src_norm, bf16) is replicated to every core
    by four quarter-AllGathers issued mid-hop (overlapped with compute)
  - per edge, the source row is fetched with dma_gather (256B bf16 rows).
    Gather descriptor generation on the Q7 cores is the machine bottleneck
    (~9ns/row on one queue pair), so gather calls are striped across the
    4 SWDGE queues: queue q runs on Q7 core pair (2q, 2q+1), and the Pool
    engine's 4-deep exec queue lets all four generate concurrently.
  - the scatter-add over destinations is a sequence of one-hot bf16 matmuls
    accumulating into a PSUM bank per 4-bin supergroup; one-hot matrices are
    built on-chip by comparing an iota row against the edge's destination
    offset
  - the DAGNN gate (sigmoid(h_k . s) * h_k) is accumulated per supergroup
    right after each hop so hop results never round-trip through HBM
Host-side numpy does the graph preprocessing once: degree norms, edge
partitioning by (destination bin, source table quarter), padding to 128-edge
chunklets uniform across cores (SPMD requires one program for all cores).
"""

import sys
import numpy as np

for _p in ("/root/.axon_site", "/root/.axon_site/_ro/trn_rl_repo", "/opt/trn_rl_repo"):
    if _p not in sys.path:
        sys.path.append(_p)

import ml_dtypes

BF16NP = np.dtype(ml_dtypes.bfloat16)

N = 100000
D = 128
K = 10
C = 8
SH = 12500          # nodes per core
NB = 98             # destination bins per core
SHP = NB * 128      # padded shard rows = 12544
NQ = 4
SG = 4              # bins per supergroup
QBINS = [30, 30, 30, 8]           # bins per table quarter; last is small so
QSTART = [0, 30, 60, 90]          # its AllGather (issued last) barely stalls
ROWSQ = [b * 128 for b in QBINS]  # per-core rows per quarter
TROWS = [C * r for r in ROWSQ]    # gathered quarter-table rows (max 26624 < int16)
FILL_OFF = 200.0    # dstoff sentinel for padding edges -> all-zero one-hot row
SINGLE_PACKET = False
QUEUE_STRIPE = True


def _wrap_idx(arr):
    """int16 index array [n] (n % 128 == 0) -> dma_gather wrapped layout
    [128, n//16]: logical i at [i % 16, i // 16], replicated to all 8 Q7
    cores (partitions 16r + i%16)."""
    n = arr.shape[0]
    w = arr.reshape(n // 16, 16).T  # [16, n//16]
    return np.tile(w, (8, 1)).astype(np.int16)


def preprocess(feats, s, src, dst):
    feats = np.asarray(feats, dtype=np.float32)
    s = np.asarray(s, dtype=np.float32).reshape(D)
    src = np.asarray(src, dtype=np.int64)
    dst = np.asarray(dst, dtype=np.int64)

    in_deg = np.bincount(dst, minlength=N).astype(np.float32)
    out_deg = np.bincount(src, minlength=N).astype(np.float32)
    dst_norm = np.clip(in_deg, 1.0, None) ** -0.5
    src_norm = np.clip(out_deg, 1.0, None) ** -0.5

    rowsq_a = np.array(ROWSQ, dtype=np.int64)
    qstart_a = np.array(QSTART, dtype=np.int64)

    owner = src // SH
    i_loc = src - owner * SH
    bs = i_loc >> 7
    q_of_edge = np.searchsorted(np.array(QSTART[1:]), bs, side="right")
    rem = owner * rowsq_a[q_of_edge] + (i_loc - 128 * qstart_a[q_of_edge])

    core_of_edge = dst // SH
    local_dst = dst - core_of_edge * SH
    bin_of_edge = local_dst >> 7
    off_of_edge = (local_dst & 127).astype(np.float32)

    per_core = []
    counts = np.zeros((C, NB, NQ), dtype=np.int64)
    for c in range(C):
        m = core_of_edge == c
        g = (bin_of_edge[m] * NQ + q_of_edge[m]).astype(np.int64)
        r = rem[m]
        o = off_of_edge[m]
        order = np.lexsort((r, g))
        g, r, o = g[order], r[order], o[order]
        cnt = np.bincount(g, minlength=NB * NQ).reshape(NB, NQ)
        counts[c] = cnt
        per_core.append((g, r, o, cnt))

    nchk = -(-counts.max(axis=0) // 128)             # [NB, NQ] chunklets
    for b in range(NB):
        if nchk[b].sum() == 0:
            nchk[b][0] = 1

    groups = [list(range(i, min(i + SG, NB))) for i in range(0, NB, SG)]

    # slot layout in stream order: for g, for q, for b in g, nchk[b][q] chunklets
    call_info = []      # (g_idx, q, slot_start, n_slots) per gather call
    binq_slot = {}
    pos = 0
    for gi, g in enumerate(groups):
        for q in range(NQ):
            call_start = pos
            for b in g:
                binq_slot[(b, q)] = pos
                pos += int(nchk[b][q]) * 128
            call_info.append((gi, q, call_start, pos - call_start))
    total_slots = pos
    ncol_chk = total_slots // 128

    idx16_all, dstoff_all = [], []
    for c in range(C):
        g_arr, r_arr, o_arr, cnt = per_core[c]
        start_of = np.zeros(NB * NQ + 1, dtype=np.int64)
        np.cumsum(cnt.reshape(-1), out=start_of[1:])
        idx_pad = np.zeros(total_slots, dtype=np.int16)
        off_pad = np.full(total_slots, FILL_OFF, dtype=np.float32)
        for b in range(NB):
            for q in range(NQ):
                n = int(cnt[b][q])
                if n == 0:
                    continue
                s0 = start_of[b * NQ + q]
                d0 = binq_slot[(b, q)]
                idx_pad[d0:d0 + n] = r_arr[s0:s0 + n].astype(np.int16)
                off_pad[d0:d0 + n] = o_arr[s0:s0 + n]
        cols = [
            _wrap_idx(idx_pad[s0:s0 + ns])
            for (_, _, s0, ns) in call_info if ns > 0
        ]
        idx16_all.append(np.concatenate(cols, axis=1))
        dstoff_all.append(np.ascontiguousarray(
            off_pad.reshape(ncol_chk, 128).T).astype(BF16NP))  # [128, ncols]

    # per-core dense inputs
    dstn_t = np.ones((C, 128, NB), dtype=np.float32)
    srcn_t = np.ones((C, 128, NB), dtype=np.float32)
    oacc0 = np.zeros((C, SHP, D), dtype=np.float32)
    hs0q = [[None] * NQ for _ in range(C)]
    for c in range(C):
        sl = slice(c * SH, (c + 1) * SH)
        dn = np.ones(SHP, dtype=np.float32)
        sn = np.ones(SHP, dtype=np.float32)
        dn[:SH] = dst_norm[sl]
        sn[:SH] = src_norm[sl]
        dstn_t[c] = dn.reshape(NB, 128).T
        srcn_t[c] = sn.reshape(NB, 128).T
        f = feats[sl]
        hs_full = np.zeros((SHP, D), dtype=np.float32)
        hs_full[:SH] = f * src_norm[sl][:, None]
        for q in range(NQ):
            r0 = 128 * QSTART[q]
            hs0q[c][q] = hs_full[r0:r0 + ROWSQ[q]].astype(BF16NP)
        gate = 1.0 / (1.0 + np.exp(-(f @ s)))
        oacc0[c, :SH] = gate[:, None] * f

    sbc = np.tile(s[None, :], (128, 1)).astype(np.float32)
    iota_f = np.tile(np.arange(128)[None, :], (128, 1)).astype(BF16NP)

    return dict(
        nchk=nchk, groups=groups, call_info=call_info, binq_slot=binq_slot,
        total_slots=total_slots, ncol_chk=ncol_chk,
        idx16=idx16_all, dstoff=dstoff_all, dstn_t=dstn_t, srcn_t=srcn_t,
        hs0q=hs0q, oacc0=oacc0, sbc=sbc, iota_f=iota_f,
    )


def build_program(pp, nhops=K):
    import concourse.bass as bass
    import concourse.mybir as mybir
    import concourse.tile as tile
    from concourse import bacc
    from concourse.bass import ds

    F32 = mybir.dt.float32
    BF = mybir.dt.bfloat16
    I16 = mybir.dt.int16
    nchk = pp["nchk"]
    groups = pp["groups"]
    binq_slot = pp["binq_slot"]
    ncol_chk = pp["ncol_chk"]

    # first/last chunklet (q, j) per bin for PSUM start/stop flags
    first_last = {}
    for b in range(NB):
        qs = [q for q in range(NQ) if nchk[b][q] > 0]
        first_last[b] = ((qs[0], 0), (qs[-1], int(nchk[b][qs[-1]]) - 1))

    # AllGather issue point: after this group index, quarter q's bins are done
    ag_after = {}
    for q in range(NQ):
        ag_after[(QSTART[q] + QBINS[q] - 1) // SG] = q

    nc = bacc.Bacc(None, num_swdge_queues=NQ if QUEUE_STRIPE else 1)
    oacc_in = nc.declare_dram_parameter("oacc0_sh", [SHP, D], F32, isOutput=False)
    idx_in = nc.declare_dram_parameter("idx16", [128, pp["total_slots"] // 16], I16, isOutput=False)
    doff_in = nc.declare_dram_parameter("dstoff", [128, ncol_chk], BF, isOutput=False)
    dstn_in = nc.declare_dram_parameter("dstn_t", [128, NB], F32, isOutput=False)
    srcn_in = nc.declare_dram_parameter("srcn_t", [128, NB], F32, isOutput=False)
    sbc_in = nc.declare_dram_parameter("sbc", [128, D], F32, isOutput=False)
    iota_in = nc.declare_dram_parameter("iota_f", [128, 128], BF, isOutput=False)
    hs0_in = [
        nc.declare_dram_parameter(f"hs0_q{q}", [ROWSQ[q], D], BF, isOutput=False)
        for q in range(NQ)
    ]
    out_d = nc.declare_dram_parameter("out_sh", [SHP, D], F32, isOutput=True)

    stages = [[nc.dram_tensor(f"hstg{k}_{q}", [ROWSQ[q], D], BF) for q in range(NQ)]
              for k in range(nhops)]
    tabs = [[nc.dram_tensor(f"tab{k}_{q}", [TROWS[q], D], BF, addr_space="Shared")
             for q in range(NQ)] for k in range(nhops)]
    rg = [list(range(C))]

    with tile.TileContext(nc) as tc:
        with tc.tile_pool(name="persist", bufs=1) as pers, \
             tc.tile_pool(name="stream", bufs=2) as st, \
             tc.tile_pool(name="idxp", bufs=3) as idxp, \
             tc.tile_pool(name="msgs", bufs=5) as msp, \
             tc.tile_pool(name="psum", bufs=1, space="PSUM") as psp:
            iota = pers.tile([128, 128], BF)
            nc.sync.dma_start(out=iota[:], in_=iota_in[:, :])
            sbc = pers.tile([128, D], F32)
            nc.sync.dma_start(out=sbc[:], in_=sbc_in[:, :])
            dstn = pers.tile([128, NB], F32)
            nc.sync.dma_start(out=dstn[:], in_=dstn_in[:, :])
            srcn = pers.tile([128, NB], F32)
            nc.sync.dma_start(out=srcn[:], in_=srcn_in[:, :])
            oacc = pers.tile([128, NB, D], F32)
            nc.sync.dma_start(
                out=oacc[:], in_=oacc_in[:, :].rearrange("(b p) d -> p b d", p=128))
            hsq = pers.tile([128, NB, D], BF)

            # initial table: hs0 quarters -> internal stage -> AllGather
            for q in range(NQ):
                nc.sync.dma_start(out=stages[0][q][:, :], in_=hs0_in[q][:, :])
                nc.gpsimd.collective_compute(
                    "AllGather", mybir.AluOpType.bypass, replica_groups=rg,
                    ins=[stages[0][q].ap().opt()], outs=[tabs[0][q].ap().opt()])

            for k in range(nhops):
                for gi, g in enumerate(groups):
                    ng = len(g)
                    # one PSUM bank per bin: a start=True matmul clears
                    # has_written for its whole bank, so bins cannot share one
                    psums = [psp.tile([128, 128], F32, name=f"ps{gi % 2}_{bl}")
                             for bl in range(ng)]
                    blks, idxts, doffs = {}, {}, {}
                    for q in range(NQ):
                        blk = sum(int(nchk[b][q]) for b in g)
                        blks[q] = blk
                        if blk == 0:
                            continue
                        s0 = binq_slot[(g[0], q)]
                        idxt = idxp.tile([128, blk * 8], I16, tag=f"idx{q}")
                        nc.sync.dma_start(
                            out=idxt[:], in_=idx_in[:, ds(s0 // 16, blk * 8)])
                        doff = st.tile([128, blk], BF, tag=f"doff{q}")
                        nc.sync.dma_start(
                            out=doff[:], in_=doff_in[:, ds(s0 // 128, blk)])
                        idxts[q], doffs[q] = idxt, doff
                    for q in range(NQ):
                        blk = blks[q]
                        if blk == 0:
                            continue
                        idxt, doff = idxts[q], doffs[q]
                        msgs = msp.tile([128, blk, 128], BF, tag="msgs")
                        nc.gpsimd.dma_gather(
                            out_ap=msgs[:],
                            in_ap=tabs[k][q][:, :],
                            idxs_ap=idxt[:],
                            num_idxs=blk * 128, num_idxs_reg=blk * 128,
                            elem_size=128, single_packet=SINGLE_PACKET,
                            queue_num=q if QUEUE_STRIPE else 0)
                        oh = st.tile([128, blk, 128], BF, tag="oh")
                        nc.vector.tensor_tensor(
                            oh[:],
                            iota[:, None, :].to_broadcast((128, blk, 128)),
                            doff[:, :, None].to_broadcast((128, blk, 128)),
                            mybir.AluOpType.is_equal)
                        lpos = 0
                        for bl, b in enumerate(g):
                            (fq, fj), (lq, lj) = first_last[b]
                            for j in range(int(nchk[b][q])):
                                nc.tensor.matmul(
                                    psums[bl][:], oh[:, lpos, :], msgs[:, lpos, :],
                                    start=(q == fq and j == fj),
                                    stop=(q == lq and j == lj))
                                lpos += 1
                    # postprocess the supergroup: norms, gate, readout accum
                    g0 = g[0]
                    hn = st.tile([128, ng, 128], F32, tag="hn")
                    for bl in range(ng):
                        nc.vector.tensor_tensor(
                            hn[:, bl, :], psums[bl][:],
                            dstn[:, g0 + bl, None].to_broadcast((128, 128)),
                            mybir.AluOpType.mult)
                    tmp = st.tile([128, ng, 128], F32, tag="tmp")
                    nc.vector.tensor_tensor(
                        tmp[:], hn[:],
                        sbc[:, None, :].to_broadcast((128, ng, 128)),
                        mybir.AluOpType.mult)
                    gcol = st.tile([128, ng, 1], F32, tag="gcol")
                    nc.vector.tensor_reduce(
                        gcol[:], tmp[:], mybir.AxisListType.X,
                        mybir.AluOpType.add)
                    sig = st.tile([128, ng, 1], F32, tag="sig")
                    nc.scalar.activation(
                        sig[:], gcol[:], mybir.ActivationFunctionType.Sigmoid)
                    contrib = st.tile([128, ng, 128], F32, tag="contrib")
                    nc.vector.tensor_tensor(
                        contrib[:], hn[:],
                        sig[:, :, :].to_broadcast((128, ng, 128)),
                        mybir.AluOpType.mult)
                    nc.vector.tensor_tensor(
                        oacc[:, g0:g0 + ng, :], oacc[:, g0:g0 + ng, :],
                        contrib[:], mybir.AluOpType.add)
                    if k < nhops - 1:
                        nc.vector.tensor_tensor(
                            hsq[:, g0:g0 + ng, :], hn[:],
                            srcn[:, g0:g0 + ng, None].to_broadcast((128, ng, 128)),
                            mybir.AluOpType.mult)
                        if gi in ag_after:
                            q = ag_after[gi]
                            nc.sync.dma_start(
                                out=stages[k + 1][q][:, :].rearrange(
                                    "(b p) d -> p b d", p=128),
                                in_=hsq[:, QSTART[q]:QSTART[q] + QBINS[q], :])
                            nc.gpsimd.collective_compute(
                                "AllGather", mybir.AluOpType.bypass,
                                replica_groups=rg,
                                ins=[stages[k + 1][q].ap().opt()],
                                outs=[tabs[k + 1][q].ap().opt()])
            nc.sync.dma_start(
                out=out_d[:, :].rearrange("(b p) d -> p b d", p=128),
                in_=oacc[:])
    nc.finalize()
    return nc


def kernel(feats, s, src, dst, nhops=K, want_profile=False, profile_dir=None):
    from concourse.bass_utils import run_bass_kernel_spmd

    pp = preprocess(feats, s, src, dst)
    nc = build_program(pp, nhops=nhops)

    in_maps = []
    for c in range(C):
        m = {
            "oacc0_sh": pp["oacc0"][c],
            "idx16": pp["idx16"][c],
            "dstoff": pp["dstoff"][c],
            "dstn_t": pp["dstn_t"][c],
            "srcn_t": pp["srcn_t"][c],
            "sbc": pp["sbc"],
            "iota_f": pp["iota_f"],
        }
        for q in range(NQ):
            m[f"hs0_q{q}"] = pp["hs0q"][c][q]
        in_maps.append(m)

    if want_profile:
        import contextlib
        from trn_agent_boot.trn_boot import _ntff_profile_via_ctypes
        from concourse import bass2jax
        hook = _ntff_profile_via_ctypes('/opt/axon/libaxon_pjrt.so')
        ctx = hook(profile_dir, [0]) if hook else contextlib.nullcontext()
        with ctx:
            results = bass2jax.run_bass_via_pjrt(nc, in_maps, n_cores=C)
    else:
        results = run_bass_kernel_spmd(nc, in_maps, list(range(C))).results

    out = np.empty((N, D), dtype=np.float32)
    for c in range(C):
        out[c * SH:(c + 1) * SH] = results[c]["out_sh"][:SH]
    return out
